# revision 20
# baseline (speedup 1.0000x reference)
"""Point-cloud rasterization + SH shading kernel for 8 Trainium2 cores.

Algorithm (dense, z-sorted):
  - Host: project points (copy) to bin them into 32 row-chunks (4 image rows
    each), z-sort, assign chunks to (core, slot) to balance load, pad lists.
    Also precompute per-pixel SH basis (from normalized viewdir) so the
    device needs no sqrt, and rescaled projection/pixel coefficient rows so
    q = R2 - d2 (the 1/R2 scale folds into activation scales).
  - Device (per core, SPMD): project its gathered points (fp32r matmul),
    compute per-(point, pixel) q = R2 - d2 via a K=4 fp32 matmul, take
    w = relu(q) (Scalar), lg = ln(1-(1-eps)w') (Scalar), ind = q>0 (DVE),
    run both exclusive cumulative sums (coverage count and log-
    transmittance) with a single strict-triangular fp32r matmul per tile
    over the combined [ind | lg] tile, accumulating the cross-tile carry in
    the same PSUM bank via an inclusive-lower-triangular fp32r matmul
    (ones = triu1 + tril0), then Tr = exp (Scalar), t1 = w*Tr (GpSimd),
    wT = (cnt<15.5)*t1 (DVE),
    composite the 30-channel features with an fp32r matmul, transpose to
    pixel-major with DVE 32x32 stream transposes, and evaluate the
    host-provided SH basis per pixel (GpSimd mults + DVE adds), clip.
  - The count cutoff is exact: for any selected point, all earlier covering
    points are also selected (inclusive counts are monotone in z), so lg
    needs no mask and the (cnt <= 16) mask is applied once at the end.
"""

import numpy as np

S = 128
N = 4096
KSEL = 16
RS = 0.03
R2 = RS * RS
F = 2.0
NCORES = 8
CHROWS = 4                 # image rows per chunk
NCHUNK = S // CHROWS       # 32
PIX = CHROWS * S           # 512 pixels per chunk
NSLOT = NCHUNK // NCORES   # 4 chunks (slots) per core
EPS = 1e-6

_C0 = 0.28209479177387814
_C1 = 0.4886025119029199
_C2 = (1.0925484305920792, -1.0925484305920792, 0.31539156525252005,
       -1.0925484305920792, 0.5462742152960396)

_BUILD_CACHE = {}


def _host_prep(vertsparam, sh_param, viewdir, cam_R, cam_T):
    v = np.asarray(vertsparam, dtype=np.float32)
    sh = np.asarray(sh_param, dtype=np.float32)
    vd = np.asarray(viewdir, dtype=np.float32)
    R = np.asarray(cam_R, dtype=np.float32)
    T = np.asarray(cam_T, dtype=np.float32)

    # host-side projection copy, used only for binning / sorting decisions
    cam = (v @ R + T).astype(np.float32)
    z = cam[:, 2]
    with np.errstate(divide="ignore", invalid="ignore"):
        x = (F * cam[:, 0] / z).astype(np.float32)
        y = (F * cam[:, 1] / z).astype(np.float32)

    order = np.argsort(z, kind="stable")
    zs, xs, ys = z[order], x[order], y[order]

    g = (1.0 - (2.0 * np.arange(S) + 1.0) / S).astype(np.float32)
    xmin, xmax = g.min() - RS, g.max() + RS

    chunk_lists = []
    chunk_maxcov = []
    for c in range(NCHUNK):
        rows = np.arange(CHROWS * c, CHROWS * c + CHROWS)
        pys = -g[rows]
        sel = ((zs > 0) & (ys >= pys.min() - RS) & (ys <= pys.max() + RS)
               & (xs >= xmin) & (xs <= xmax))
        pts = np.where(sel)[0]
        chunk_lists.append(order[pts])
        if len(pts):
            px = np.tile(g, CHROWS)
            py = np.repeat(-g[rows], S)
            d2 = ((px[:, None] - xs[pts][None, :]) ** 2
                  + (py[:, None] - ys[pts][None, :]) ** 2)
            chunk_maxcov.append(int((d2 < R2).sum(1).max()))
        else:
            chunk_maxcov.append(0)
    cnts = np.array([len(l) for l in chunk_lists])

    rank = np.argsort(-cnts, kind="stable")
    slots = [[int(rank[NCORES * s + k]) for k in range(NCORES)]
             for s in range(NSLOT)]
    L = [max(128, int(np.ceil(max(cnts[c] for c in slots[s]) / 128)) * 128)
         for s in range(NSLOT)]
    P_tot = sum(L)
    # per-slot: does any chunk in the slot have a pixel covered by >16 points?
    need = tuple(bool(max(chunk_maxcov[c] for c in slots[s]) > KSEL)
                 for s in range(NSLOT))

    # pad vertex that projects far off-screen with z=1 (w == 0 everywhere)
    cam_pad = np.array([1e4, 1e4, 1.0], dtype=np.float64)
    v_pad = np.linalg.solve(R.astype(np.float64).T,
                            (cam_pad - T.astype(np.float64))).astype(np.float32)

    in_maps = []
    meta = []  # per core: list of chunk ids per slot
    for k in range(NCORES):
        verts_g = np.tile(v_pad, (P_tot, 1)).astype(np.float32)
        feats_g = np.zeros((P_tot, 30), dtype=np.float32)
        pixrhs = np.zeros((4, NSLOT * PIX), dtype=np.float32)
        basis = np.zeros((128, NSLOT * CHROWS * 9), dtype=np.float32)
        off = 0
        chunks_k = []
        for s in range(NSLOT):
            c = slots[s][k]
            chunks_k.append(c)
            pts = chunk_lists[c]
            n = len(pts)
            verts_g[off:off + n] = v[pts]
            feats_g[off:off + n] = sh[pts]
            rows = np.arange(CHROWS * c, CHROWS * c + CHROWS)
            px = np.tile(g, CHROWS)
            py = np.repeat(-g[rows], S)
            pixrhs[0, s * PIX:(s + 1) * PIX] = px
            pixrhs[1, s * PIX:(s + 1) * PIX] = py
            pixrhs[2, s * PIX:(s + 1) * PIX] = -(px * px + py * py)
            pixrhs[3, s * PIX:(s + 1) * PIX] = 1.0
            # basis[col, (s, g, b)] for normalized viewdir of pixel (row, col)
            d = vd[rows].transpose(1, 0, 2).astype(np.float64)  # [128, 4, 3]
            d = d / np.linalg.norm(d, axis=-1, keepdims=True)
            dx, dy, dz = d[..., 0], d[..., 1], d[..., 2]
            B = np.stack([
                -_C1 * dy, _C1 * dz, -_C1 * dx,
                _C2[0] * dx * dy, _C2[1] * dy * dz,
                _C2[2] * (2.0 * dz * dz - dx * dx - dy * dy),
                _C2[3] * dx * dz, _C2[4] * (dx * dx - dy * dy),
            ], axis=-1)  # [128, 4, 8]
            B = np.concatenate([B, np.zeros((128, CHROWS, 1))], axis=-1)
            basis[:, (s * CHROWS) * 9:(s * CHROWS + CHROWS) * 9] = (
                B.reshape(128, CHROWS * 9).astype(np.float32))
            off += L[s]
        in_maps.append({
            "vertsT": np.ascontiguousarray(verts_g.T),        # [3, P_tot]
            "feats": np.ascontiguousarray(feats_g),           # [P_tot, 30]
            "pixrhs": np.ascontiguousarray(pixrhs),           # [4, 2048]
            "basis": np.ascontiguousarray(basis),             # [128, 144]
            "camR": np.ascontiguousarray(R),                  # [3, 3]
            "camTc": np.ascontiguousarray(T.reshape(3, 1)),   # [3, 1]
            "triu1": np.triu(np.ones((128, 128), dtype=np.float32), 1),
            "tril0": np.tril(np.ones((128, 128), dtype=np.float32), 0),
        })
        meta.append(chunks_k)
    return tuple(L), need, in_maps, meta


def _build(L, need):
    from contextlib import ExitStack

    import concourse.bacc as bacc
    import concourse.bass as bass
    import concourse.hw_specs as hw_specs
    import concourse.tile as tile
    from concourse import mybir

    f32 = mybir.dt.float32
    f32r = mybir.dt.float32r
    Act = mybir.ActivationFunctionType
    Alu = mybir.AluOpType

    P_tot = sum(L)
    ntiles = [l // 128 for l in L]
    tile_base = np.cumsum([0] + ntiles).tolist()
    NT = sum(ntiles)
    PF = P_tot // 128

    nc = bacc.Bacc(None, target_bir_lowering=False)

    d_vertsT = nc.dram_tensor("vertsT", [3, P_tot], f32r, kind="ExternalInput")
    d_feats = nc.dram_tensor("feats", [P_tot, 30], f32r, kind="ExternalInput")
    d_pixrhs = nc.dram_tensor("pixrhs", [4, NSLOT * PIX], f32, kind="ExternalInput")
    d_basis = nc.dram_tensor("basis", [128, NSLOT * CHROWS * 9], f32,
                             kind="ExternalInput")
    d_R = nc.dram_tensor("camR", [3, 3], f32r, kind="ExternalInput")
    d_Tc = nc.dram_tensor("camTc", [3, 1], f32, kind="ExternalInput")
    d_triu1 = nc.dram_tensor("triu1", [128, 128], f32r, kind="ExternalInput")
    d_tril0 = nc.dram_tensor("tril0", [128, 128], f32r, kind="ExternalInput")
    d_out = nc.dram_tensor("out", [128, NSLOT * CHROWS * 3], f32,
                           kind="ExternalOutput")

    def bcast_free(ap, count):
        # broadcast a [..., 1]-style AP along a new innermost free dim
        return bass.AP(tensor=ap.tensor, offset=ap.offset,
                       ap=list(ap.ap) + [[0, count]])

    with tile.TileContext(nc) as tc, ExitStack() as ctx:
        consts = ctx.enter_context(tc.tile_pool(name="consts", bufs=1))

        vertsT = consts.tile([3, P_tot], f32r)
        nc.sync.dma_start(out=vertsT, in_=d_vertsT[:])
        pixrhs = consts.tile([4, NSLOT * PIX], f32)
        nc.sync.dma_start(out=pixrhs, in_=d_pixrhs[:])
        feats = consts.tile([128, NT, 30], f32r)
        nc.sync.dma_start(
            out=feats, in_=d_feats.rearrange("(t p) c -> p t c", p=128))
        basis = consts.tile([128, NSLOT, CHROWS, 9], f32)
        nc.sync.dma_start(out=basis, in_=d_basis[:].rearrange(
            "p (s g b) -> p s g b", s=NSLOT, g=CHROWS))
        R_sb = consts.tile([3, 3], f32r)
        nc.sync.dma_start(out=R_sb, in_=d_R[:])
        T_sb = consts.tile([3, 1], f32)
        nc.sync.dma_start(out=T_sb, in_=d_Tc[:])
        triu1 = consts.tile([128, 128], f32r)
        nc.sync.dma_start(out=triu1, in_=d_triu1[:])
        tril0 = consts.tile([128, 128], f32r)
        nc.sync.dma_start(out=tril0, in_=d_tril0[:])

        # ---- projection: camT = R^T @ vertsT + T, then pcoef rows ----
        # Compute engines may only address partition offsets {0,32,64,96},
        # so every row lives on partition 0 of its own tile; DMA (which has
        # no such restriction) extracts camT rows and assembles pcoef.
        # pcoef rows: {2*x_ndc, 2*y_ndc, 1, R2 - x_ndc^2 - y_ndc^2} so that
        # q = pcoef . {px, py, -(px^2+py^2), 1} = R2 - d2.
        camT = consts.tile([3, P_tot], f32)
        pcoef = consts.tile([4, P_tot], f32)
        xrow = consts.tile([128, PF], f32)
        yrow = consts.tile([128, PF], f32)
        zrow = consts.tile([128, PF], f32)
        rz = consts.tile([128, PF], f32)
        r0 = consts.tile([128, PF], f32)
        r1 = consts.tile([128, PF], f32)
        s0 = consts.tile([128, PF], f32)
        s1 = consts.tile([128, PF], f32)
        onesrow = consts.tile([128, PF], f32)
        nc.vector.memset(onesrow, 1.0)

        with tc.tile_pool(name="pproj", bufs=2, space="PSUM") as pproj:
            nchk = (P_tot + 511) // 512
            for i in range(nchk):
                a, b = 512 * i, min(512 * (i + 1), P_tot)
                pt = pproj.tile([3, 512], f32)
                nc.tensor.matmul(pt[:, :b - a], R_sb[:],
                                 vertsT[:, a:b], start=True, stop=True)
                nc.vector.tensor_scalar(camT[:, a:b], pt[:, :b - a],
                                        T_sb[:], None, Alu.add)

        nc.sync.dma_start(out=xrow, in_=camT[0:1, :])
        nc.sync.dma_start(out=yrow, in_=camT[1:2, :])
        nc.sync.dma_start(out=zrow, in_=camT[2:3, :])
        nc.vector.reciprocal(rz, zrow)
        nc.vector.tensor_scalar(xrow, xrow, float(2.0 * F), None, Alu.mult)
        nc.vector.tensor_scalar(yrow, yrow, float(2.0 * F), None, Alu.mult)
        nc.vector.tensor_mul(r0, xrow, rz)      # 2*x_ndc
        nc.vector.tensor_mul(r1, yrow, rz)      # 2*y_ndc
        nc.vector.tensor_mul(s0, r0, r0)
        nc.vector.tensor_mul(s1, r1, r1)
        nc.vector.tensor_add(s0, s0, s1)        # 4*(xn^2+yn^2)
        nc.vector.tensor_scalar(s0, s0, -0.25, float(R2), Alu.mult, Alu.add)
        nc.sync.dma_start(out=pcoef[0:1, :], in_=r0)
        nc.sync.dma_start(out=pcoef[1:2, :], in_=r1)
        nc.sync.dma_start(out=pcoef[2:3, :], in_=onesrow)
        nc.sync.dma_start(out=pcoef[3:4, :], in_=s0)

        outsb = consts.tile([128, NSLOT, CHROWS, 3], f32)

        wrk = ctx.enter_context(tc.tile_pool(name="wrk", bufs=6))
        xpool = ctx.enter_context(tc.tile_pool(name="xpool", bufs=6))
        shp = ctx.enter_context(tc.tile_pool(name="shp", bufs=2))
        tmpp = ctx.enter_context(tc.tile_pool(name="tmpp", bufs=4))
        pq = ctx.enter_context(tc.tile_pool(name="pq", bufs=2, space="PSUM"))
        pXa = ctx.enter_context(tc.tile_pool(name="pXa", bufs=1, space="PSUM"))
        pXb = ctx.enter_context(tc.tile_pool(name="pXb", bufs=1, space="PSUM"))
        # matmul PSUM outputs must sit at partition 0, so each stream gets
        # its own one-bank image accumulator; stream B's slots run
        # sequentially and recycle theirs (WAR tracked by the pool).
        pimgA = ctx.enter_context(tc.tile_pool(name="pimgA", bufs=1, space="PSUM"))
        pimgB = ctx.enter_context(tc.tile_pool(name="pimgB", bufs=1, space="PSUM"))

        # two interleaved tile streams: slot 0 alone, slots 1..3 in sequence
        streams = [[(0, t) for t in range(ntiles[0])],
                   [(s, t) for s in range(1, NSLOT) for t in range(ntiles[s])]]
        sched = []
        i = j = 0
        while i < len(streams[0]) or j < len(streams[1]):
            if i < len(streams[0]):
                sched.append(streams[0][i]); i += 1
            if j < len(streams[1]):
                sched.append(streams[1][j]); j += 1

        Xp = {}
        imgT = {}
        wbuf = {}
        Xbuf = {}

        def emit_epilogue(s):
            # imgT[s] holds this slot's composited channels (x R2).
            imgTs = shp.tile([32, PIX], f32, tag="imgTs")
            # w carries a factor R2 (q = R2 - d2); undo it on the way out
            nc.scalar.activation(imgTs[0:30, :], imgT[s][:], Act.Copy,
                                 scale=float(1.0 / R2))
            v32 = imgTs[:, :].rearrange("p (g j c) -> p g j c",
                                        g=CHROWS, j=4)
            img30 = shp.tile([128, CHROWS, 32], f32, tag="img30")
            for jb in range(4):
                nc.vector.transpose(img30[32 * jb:32 * (jb + 1)],
                                    v32[:, :, jb, :])
            sh30 = img30[:, :, 0:30].rearrange("p g (b c) -> p g b c", b=10)
            acc = shp.tile([128, CHROWS, 3], f32, tag="acc")
            nc.vector.scalar_tensor_tensor(acc, sh30[:, :, 1, :], float(_C0),
                                           sh30[:, :, 0, :], Alu.mult, Alu.add)
            for b in range(8):
                tmp = tmpp.tile([128, CHROWS, 3], f32, tag=f"tmp{b}")
                nc.vector.tensor_tensor(tmp, sh30[:, :, b + 2, :],
                                        bcast_free(basis[:, s, :, b], 3),
                                        Alu.mult)
                nc.vector.tensor_add(acc, acc, tmp)
            nc.vector.tensor_scalar(outsb[:, s], acc, 0.0, 1.0,
                                    Alu.max, Alu.min)

        def emit_a(s, t):
            nt = ntiles[s]
            gt = tile_base[s] + t
            toff = 128 * gt
            xw = 2 * PIX if need[s] else PIX      # [ind | lg] or [lg]
            lgo = PIX if need[s] else 0
            rhs_pix = pixrhs[:, s * PIX:(s + 1) * PIX]

            if t == 0:
                pool = pXa if s == 0 else pXb
                Xp[s] = pool.tile([128, 2 * PIX], f32,
                                  tag="Xpa" if s == 0 else "Xpb",
                                  name=f"Xp{s}")[:, 0:xw]
                ipool = pimgA if s == 0 else pimgB
                imgT[s] = ipool.tile([30, PIX], f32,
                                     tag="imgA" if s == 0 else "imgB",
                                     name=f"imgT{s}")

            q = pq.tile([128, PIX], f32, tag="q")
            nc.tensor.matmul(q, pcoef[:, toff:toff + 128], rhs_pix,
                             start=True, stop=True)
            w = wrk.tile([128, PIX], f32, tag="w", name=f"w{s}_{t}")
            nc.scalar.activation(w, q, Act.Relu)
            X = xpool.tile([128, xw], f32r, tag="X", name=f"X{s}_{t}")
            if need[s]:
                nc.vector.tensor_scalar(X[:, 0:PIX], q, 0.0, None, Alu.is_gt)
            nc.scalar.activation(X[:, lgo:lgo + PIX], w, Act.Ln,
                                 bias=1.0, scale=float(-(1.0 - EPS) / R2))
            wbuf[(s, t)] = w
            Xbuf[(s, t)] = X

        def emit_b(s, t):
            nt = ntiles[s]
            gt = tile_base[s] + t
            lgo = PIX if need[s] else 0
            w = wbuf.pop((s, t))
            X = Xbuf.pop((s, t))

            # strict-upper cumsum: exclusive coverage count and exclusive
            # log-transmittance (plus the carry already in the bank).
            # One matmul per 512-wide PSUM bank (a single matmul may not
            # span two banks).
            if need[s]:
                nc.tensor.matmul(Xp[s][:, 0:PIX], triu1[:], X[:, 0:PIX],
                                 start=(t == 0), stop=(t == nt - 1),
                                 skip_group_check=True)
            nc.tensor.matmul(Xp[s][:, lgo:lgo + PIX], triu1[:],
                             X[:, lgo:lgo + PIX],
                             start=(t == 0), stop=(t == nt - 1),
                             skip_group_check=True)
            Tr = wrk.tile([128, PIX], f32, tag="Tr")
            nc.scalar.activation(Tr, Xp[s][:, lgo:lgo + PIX], Act.Exp)
            t1 = wrk.tile([128, PIX], f32r, tag="t1")
            nc.gpsimd.tensor_tensor(t1, w, Tr, Alu.mult)
            if need[s]:
                wT = wrk.tile([128, PIX], f32r, tag="wT")
                nc.vector.scalar_tensor_tensor(wT, Xp[s][:, 0:PIX],
                                               float(KSEL) - 0.5, t1,
                                               Alu.is_lt, Alu.mult)
            else:
                wT = t1
            nc.tensor.matmul(imgT[s][:], feats[:, gt, :],
                             wT[:], start=(t == 0), stop=(t == nt - 1),
                             skip_group_check=True)
            if t < nt - 1:
                # promote the bank to the next tile's carry:
                # += tril0 @ X makes every row hold the full column sum
                # (triu1 + tril0 == ones)
                if need[s]:
                    nc.tensor.matmul(Xp[s][:, 0:PIX], tril0[:], X[:, 0:PIX],
                                     start=False, stop=False,
                                     skip_group_check=True)
                nc.tensor.matmul(Xp[s][:, lgo:lgo + PIX], tril0[:],
                                 X[:, lgo:lgo + PIX],
                                 start=False, stop=False,
                                 skip_group_check=True)
            if t == nt - 1:
                emit_epilogue(s)

        # run phase A a few tiles ahead of phase B so the PE queue stays
        # dense (ramps the p-state) and never stalls on Scalar/DVE
        DELAY = 4
        for i, (s, t) in enumerate(sched):
            emit_a(s, t)
            if i >= DELAY:
                emit_b(*sched[i - DELAY])
        for i in range(max(0, len(sched) - DELAY), len(sched)):
            emit_b(*sched[i])

        nc.sync.dma_start(
            out=d_out[:],
            in_=outsb.rearrange("p s g c -> p (s g c)"))

    # The act-table placement pass picks a table per activation locally,
    # flapping between the ln-only and exp-only sets (one 1.3us load per
    # flap).  ln and exp share the natural_log_exp_and_others set; hide
    # them from every other set during placement so the pass settles on
    # the shared one (walrus still loads the full real table contents).
    orig_get = hw_specs.get_activation_tables

    def _pinned(arch):
        tabs = orig_get(arch)
        out = {}
        for name, funcs in tabs.items():
            if name != "natural_log_exp_and_others":
                funcs = {f for f in funcs
                         if f.name.lower() not in ("ln", "exp")}
            out[name] = funcs
        return out

    hw_specs.get_activation_tables = _pinned
    try:
        nc.compile()
    finally:
        hw_specs.get_activation_tables = orig_get
    return nc


def kernel(vertsparam, sh_param, viewdir, cam_R, cam_T, _trace=False):
    from concourse.bass_utils import run_bass_kernel_spmd

    L, need, in_maps, meta = _host_prep(vertsparam, sh_param, viewdir,
                                        cam_R, cam_T)
    key = (L, need)
    if key not in _BUILD_CACHE:
        _BUILD_CACHE[key] = _build(L, need)
    nc = _BUILD_CACHE[key]

    res = run_bass_kernel_spmd(nc, in_maps, core_ids=list(range(NCORES)),
                               trace=_trace)

    image = np.zeros((1, S, S, 3), dtype=np.float32)
    for k in range(NCORES):
        out = res.results[k]["out"].reshape(128, NSLOT, CHROWS, 3)
        for s in range(NSLOT):
            c = meta[k][s]
            for gi in range(CHROWS):
                image[0, CHROWS * c + gi, :, :] = out[:, s, gi, :]
    if _trace:
        kernel._last_exec_time_ns = res.exec_time_ns
        kernel._last_trace = res.instructions_and_trace
    return image


# revision 21
# speedup vs baseline: 1.2525x; 1.2525x over previous
"""Point-cloud rasterization + SH shading kernel for 8 Trainium2 cores.

Algorithm (dense, z-sorted):
  - Host: project points (copy) to bin them into 32 row-chunks (4 image rows
    each), z-sort, assign chunks to (core, slot) to balance load, pad lists.
    Also precompute per-pixel SH basis (from normalized viewdir) so the
    device needs no sqrt, and rescaled projection/pixel coefficient rows so
    q = R2 - d2 (the 1/R2 scale folds into activation scales).
  - Device (per core, SPMD): project its gathered points (fp32r matmul),
    compute per-(point, pixel) q = R2 - d2 via a K=4 fp32 matmul, take
    w = relu(q) (Scalar), lg = ln(1-(1-eps)w') (Scalar), ind = q>0 (DVE),
    run both exclusive cumulative sums (coverage count and log-
    transmittance) with a single strict-triangular fp32r matmul per tile
    over the combined [ind | lg] tile, accumulating the cross-tile carry in
    the same PSUM bank via an inclusive-lower-triangular fp32r matmul
    (ones = triu1 + tril0), then Tr = exp (Scalar), t1 = w*Tr (GpSimd),
    wT = (cnt<15.5)*t1 (DVE),
    composite the 30-channel features with an fp32r matmul, transpose to
    pixel-major with DVE 32x32 stream transposes, and evaluate the
    host-provided SH basis per pixel (GpSimd mults + DVE adds), clip.
  - The count cutoff is exact: for any selected point, all earlier covering
    points are also selected (inclusive counts are monotone in z), so lg
    needs no mask and the (cnt <= 16) mask is applied once at the end.
"""

import numpy as np

S = 128
N = 4096
KSEL = 16
RS = 0.03
R2 = RS * RS
F = 2.0
NCORES = 8
CHROWS = 4                 # image rows per chunk
NCHUNK = S // CHROWS       # 32
PIX = CHROWS * S           # 512 pixels per chunk
NSLOT = NCHUNK // NCORES   # 4 chunks (slots) per core
EPS = 1e-6

_C0 = 0.28209479177387814
_C1 = 0.4886025119029199
_C2 = (1.0925484305920792, -1.0925484305920792, 0.31539156525252005,
       -1.0925484305920792, 0.5462742152960396)

_BUILD_CACHE = {}


def _host_prep(vertsparam, sh_param, viewdir, cam_R, cam_T):
    v = np.asarray(vertsparam, dtype=np.float32)
    sh = np.asarray(sh_param, dtype=np.float32)
    vd = np.asarray(viewdir, dtype=np.float32)
    R = np.asarray(cam_R, dtype=np.float32)
    T = np.asarray(cam_T, dtype=np.float32)

    # host-side projection copy, used only for binning / sorting decisions
    cam = (v @ R + T).astype(np.float32)
    z = cam[:, 2]
    with np.errstate(divide="ignore", invalid="ignore"):
        x = (F * cam[:, 0] / z).astype(np.float32)
        y = (F * cam[:, 1] / z).astype(np.float32)

    order = np.argsort(z, kind="stable")
    zs, xs, ys = z[order], x[order], y[order]

    g = (1.0 - (2.0 * np.arange(S) + 1.0) / S).astype(np.float32)
    xmin, xmax = g.min() - RS, g.max() + RS

    chunk_lists = []
    chunk_maxcov = []
    for c in range(NCHUNK):
        rows = np.arange(CHROWS * c, CHROWS * c + CHROWS)
        pys = -g[rows]
        sel = ((zs > 0) & (ys >= pys.min() - RS) & (ys <= pys.max() + RS)
               & (xs >= xmin) & (xs <= xmax))
        pts = np.where(sel)[0]
        chunk_lists.append(order[pts])
        if len(pts):
            px = np.tile(g, CHROWS)
            py = np.repeat(-g[rows], S)
            d2 = ((px[:, None] - xs[pts][None, :]) ** 2
                  + (py[:, None] - ys[pts][None, :]) ** 2)
            chunk_maxcov.append(int((d2 < R2).sum(1).max()))
        else:
            chunk_maxcov.append(0)
    cnts = np.array([len(l) for l in chunk_lists])

    rank = np.argsort(-cnts, kind="stable")
    slots = [[int(rank[NCORES * s + k]) for k in range(NCORES)]
             for s in range(NSLOT)]
    L = [max(128, int(np.ceil(max(cnts[c] for c in slots[s]) / 128)) * 128)
         for s in range(NSLOT)]
    P_tot = sum(L)
    # per-slot: does any chunk in the slot have a pixel covered by >16 points?
    need = tuple(bool(max(chunk_maxcov[c] for c in slots[s]) > KSEL)
                 for s in range(NSLOT))

    # pad vertex that projects far off-screen with z=1 (w == 0 everywhere)
    cam_pad = np.array([1e4, 1e4, 1.0], dtype=np.float64)
    v_pad = np.linalg.solve(R.astype(np.float64).T,
                            (cam_pad - T.astype(np.float64))).astype(np.float32)

    in_maps = []
    meta = []  # per core: list of chunk ids per slot
    for k in range(NCORES):
        verts_g = np.tile(v_pad, (P_tot, 1)).astype(np.float32)
        feats_g = np.zeros((P_tot, 30), dtype=np.float32)
        pixrhs = np.zeros((4, NSLOT * PIX), dtype=np.float32)
        basis = np.zeros((128, NSLOT * CHROWS * 9), dtype=np.float32)
        off = 0
        chunks_k = []
        for s in range(NSLOT):
            c = slots[s][k]
            chunks_k.append(c)
            pts = chunk_lists[c]
            n = len(pts)
            verts_g[off:off + n] = v[pts]
            feats_g[off:off + n] = sh[pts]
            rows = np.arange(CHROWS * c, CHROWS * c + CHROWS)
            px = np.tile(g, CHROWS)
            py = np.repeat(-g[rows], S)
            pixrhs[0, s * PIX:(s + 1) * PIX] = px
            pixrhs[1, s * PIX:(s + 1) * PIX] = py
            pixrhs[2, s * PIX:(s + 1) * PIX] = -(px * px + py * py)
            pixrhs[3, s * PIX:(s + 1) * PIX] = 1.0
            # basis[col, (s, g, b)] for normalized viewdir of pixel (row, col)
            d = vd[rows].transpose(1, 0, 2).astype(np.float64)  # [128, 4, 3]
            d = d / np.linalg.norm(d, axis=-1, keepdims=True)
            dx, dy, dz = d[..., 0], d[..., 1], d[..., 2]
            B = np.stack([
                -_C1 * dy, _C1 * dz, -_C1 * dx,
                _C2[0] * dx * dy, _C2[1] * dy * dz,
                _C2[2] * (2.0 * dz * dz - dx * dx - dy * dy),
                _C2[3] * dx * dz, _C2[4] * (dx * dx - dy * dy),
            ], axis=-1)  # [128, 4, 8]
            B = np.concatenate([B, np.zeros((128, CHROWS, 1))], axis=-1)
            basis[:, (s * CHROWS) * 9:(s * CHROWS + CHROWS) * 9] = (
                B.reshape(128, CHROWS * 9).astype(np.float32))
            off += L[s]
        in_maps.append({
            "vertsT": np.ascontiguousarray(verts_g.T),        # [3, P_tot]
            "feats": np.ascontiguousarray(feats_g),           # [P_tot, 30]
            "pixrhs": np.ascontiguousarray(pixrhs),           # [4, 2048]
            "basis": np.ascontiguousarray(basis),             # [128, 144]
            "camR": np.ascontiguousarray(R),                  # [3, 3]
            "camTc": np.ascontiguousarray(T.reshape(3, 1)),   # [3, 1]
            "triu1": np.triu(np.ones((128, 128), dtype=np.float32), 1),
            "tril0": np.tril(np.ones((128, 128), dtype=np.float32), 0),
        })
        meta.append(chunks_k)
    return tuple(L), need, in_maps, meta


def _build(L, need):
    from contextlib import ExitStack

    import concourse.bacc as bacc
    import concourse.bass as bass
    import concourse.hw_specs as hw_specs
    import concourse.tile as tile
    from concourse import mybir

    f32 = mybir.dt.float32
    f32r = mybir.dt.float32r
    Act = mybir.ActivationFunctionType
    Alu = mybir.AluOpType

    P_tot = sum(L)
    ntiles = [l // 128 for l in L]
    tile_base = np.cumsum([0] + ntiles).tolist()
    NT = sum(ntiles)
    PF = P_tot // 128

    nc = bacc.Bacc(None, target_bir_lowering=False)

    d_vertsT = nc.dram_tensor("vertsT", [3, P_tot], f32r, kind="ExternalInput")
    d_feats = nc.dram_tensor("feats", [P_tot, 30], f32r, kind="ExternalInput")
    d_pixrhs = nc.dram_tensor("pixrhs", [4, NSLOT * PIX], f32, kind="ExternalInput")
    d_basis = nc.dram_tensor("basis", [128, NSLOT * CHROWS * 9], f32,
                             kind="ExternalInput")
    d_R = nc.dram_tensor("camR", [3, 3], f32r, kind="ExternalInput")
    d_Tc = nc.dram_tensor("camTc", [3, 1], f32, kind="ExternalInput")
    d_triu1 = nc.dram_tensor("triu1", [128, 128], f32r, kind="ExternalInput")
    d_tril0 = nc.dram_tensor("tril0", [128, 128], f32r, kind="ExternalInput")
    d_out = nc.dram_tensor("out", [128, NSLOT * CHROWS * 3], f32,
                           kind="ExternalOutput")

    def bcast_free(ap, count):
        # broadcast a [..., 1]-style AP along a new innermost free dim
        return bass.AP(tensor=ap.tensor, offset=ap.offset,
                       ap=list(ap.ap) + [[0, count]])

    with tile.TileContext(nc) as tc, ExitStack() as ctx:
        consts = ctx.enter_context(tc.tile_pool(name="consts", bufs=1))

        vertsT = consts.tile([3, P_tot], f32r)
        nc.sync.dma_start(out=vertsT, in_=d_vertsT[:])
        pixrhs = consts.tile([4, NSLOT * PIX], f32)
        nc.sync.dma_start(out=pixrhs, in_=d_pixrhs[:])
        feats = consts.tile([128, NT, 30], f32r)
        nc.sync.dma_start(
            out=feats, in_=d_feats.rearrange("(t p) c -> p t c", p=128))
        basis = consts.tile([128, NSLOT, CHROWS, 9], f32)
        nc.sync.dma_start(out=basis, in_=d_basis[:].rearrange(
            "p (s g b) -> p s g b", s=NSLOT, g=CHROWS))
        R_sb = consts.tile([3, 3], f32r)
        nc.sync.dma_start(out=R_sb, in_=d_R[:])
        T_sb = consts.tile([3, 1], f32)
        nc.sync.dma_start(out=T_sb, in_=d_Tc[:])
        triu1 = consts.tile([128, 128], f32r)
        nc.sync.dma_start(out=triu1, in_=d_triu1[:])
        tril0 = consts.tile([128, 128], f32r)
        nc.sync.dma_start(out=tril0, in_=d_tril0[:])

        # ---- projection: camT = R^T @ vertsT + T, then pcoef rows ----
        # Compute engines may only address partition offsets {0,32,64,96},
        # so every row lives on partition 0 of its own tile; DMA (which has
        # no such restriction) extracts camT rows and assembles pcoef.
        # pcoef rows: {2*x_ndc, 2*y_ndc, 1, R2 - x_ndc^2 - y_ndc^2} so that
        # q = pcoef . {px, py, -(px^2+py^2), 1} = R2 - d2.
        camT = consts.tile([3, P_tot], f32)
        pcoef = consts.tile([4, P_tot], f32)
        xrow = consts.tile([128, PF], f32)
        yrow = consts.tile([128, PF], f32)
        zrow = consts.tile([128, PF], f32)
        rz = consts.tile([128, PF], f32)
        r0 = consts.tile([128, PF], f32)
        r1 = consts.tile([128, PF], f32)
        s0 = consts.tile([128, PF], f32)
        s1 = consts.tile([128, PF], f32)
        onesrow = consts.tile([128, PF], f32)
        nc.vector.memset(onesrow, 1.0)

        with tc.tile_pool(name="pproj", bufs=2, space="PSUM") as pproj:
            nchk = (P_tot + 511) // 512
            for i in range(nchk):
                a, b = 512 * i, min(512 * (i + 1), P_tot)
                pt = pproj.tile([3, 512], f32)
                nc.tensor.matmul(pt[:, :b - a], R_sb[:],
                                 vertsT[:, a:b], start=True, stop=True)
                nc.vector.tensor_scalar(camT[:, a:b], pt[:, :b - a],
                                        T_sb[:], None, Alu.add)

        nc.sync.dma_start(out=xrow, in_=camT[0:1, :])
        nc.sync.dma_start(out=yrow, in_=camT[1:2, :])
        nc.sync.dma_start(out=zrow, in_=camT[2:3, :])
        nc.vector.reciprocal(rz, zrow)
        nc.vector.tensor_scalar(xrow, xrow, float(2.0 * F), None, Alu.mult)
        nc.vector.tensor_scalar(yrow, yrow, float(2.0 * F), None, Alu.mult)
        nc.vector.tensor_mul(r0, xrow, rz)      # 2*x_ndc
        nc.vector.tensor_mul(r1, yrow, rz)      # 2*y_ndc
        nc.vector.tensor_mul(s0, r0, r0)
        nc.vector.tensor_mul(s1, r1, r1)
        nc.vector.tensor_add(s0, s0, s1)        # 4*(xn^2+yn^2)
        nc.vector.tensor_scalar(s0, s0, -0.25, float(R2), Alu.mult, Alu.add)
        nc.sync.dma_start(out=pcoef[0:1, :], in_=r0)
        nc.sync.dma_start(out=pcoef[1:2, :], in_=r1)
        nc.sync.dma_start(out=pcoef[2:3, :], in_=onesrow)
        nc.sync.dma_start(out=pcoef[3:4, :], in_=s0)

        outsb = consts.tile([128, NSLOT, CHROWS, 3], f32)

        wrk = ctx.enter_context(tc.tile_pool(name="wrk", bufs=6))
        xpool = ctx.enter_context(tc.tile_pool(name="xpool", bufs=6))
        shp = ctx.enter_context(tc.tile_pool(name="shp", bufs=2))
        tmpp = ctx.enter_context(tc.tile_pool(name="tmpp", bufs=4))
        pq = ctx.enter_context(tc.tile_pool(name="pq", bufs=2, space="PSUM"))
        pXa = ctx.enter_context(tc.tile_pool(name="pXa", bufs=1, space="PSUM"))
        pXb = ctx.enter_context(tc.tile_pool(name="pXb", bufs=1, space="PSUM"))
        # matmul PSUM outputs must sit at partition 0, so each stream gets
        # its own one-bank image accumulator; stream B's slots run
        # sequentially and recycle theirs (WAR tracked by the pool).
        pimgA = ctx.enter_context(tc.tile_pool(name="pimgA", bufs=1, space="PSUM"))
        pimgB = ctx.enter_context(tc.tile_pool(name="pimgB", bufs=1, space="PSUM"))

        # two interleaved tile streams: slot 0 alone, slots 1..3 in sequence
        streams = [[(0, t) for t in range(ntiles[0])],
                   [(s, t) for s in range(1, NSLOT) for t in range(ntiles[s])]]
        sched = []
        i = j = 0
        while i < len(streams[0]) or j < len(streams[1]):
            if i < len(streams[0]):
                sched.append(streams[0][i]); i += 1
            if j < len(streams[1]):
                sched.append(streams[1][j]); j += 1

        Xp = {}
        imgT = {}
        wbuf = {}
        Xbuf = {}

        def emit_epilogue(s):
            # imgT[s] holds this slot's composited channels (x R2).
            imgTs = shp.tile([32, PIX], f32, tag="imgTs")
            # w carries a factor R2 (q = R2 - d2); undo it on the way out
            nc.scalar.activation(imgTs[0:30, :], imgT[s][:], Act.Copy,
                                 scale=float(1.0 / R2))
            v32 = imgTs[:, :].rearrange("p (g j c) -> p g j c",
                                        g=CHROWS, j=4)
            img30 = shp.tile([128, CHROWS, 32], f32, tag="img30")
            for jb in range(4):
                nc.vector.transpose(img30[32 * jb:32 * (jb + 1)],
                                    v32[:, :, jb, :])
            sh30 = img30[:, :, 0:30].rearrange("p g (b c) -> p g b c", b=10)
            acc = shp.tile([128, CHROWS, 3], f32, tag="acc")
            nc.vector.scalar_tensor_tensor(acc, sh30[:, :, 1, :], float(_C0),
                                           sh30[:, :, 0, :], Alu.mult, Alu.add)
            for b in range(8):
                tmp = tmpp.tile([128, CHROWS, 3], f32, tag=f"tmp{b}")
                nc.vector.tensor_tensor(tmp, sh30[:, :, b + 2, :],
                                        bcast_free(basis[:, s, :, b], 3),
                                        Alu.mult)
                nc.vector.tensor_add(acc, acc, tmp)
            nc.vector.tensor_scalar(outsb[:, s], acc, 0.0, 1.0,
                                    Alu.max, Alu.min)

        def emit_a(s, t):
            nt = ntiles[s]
            gt = tile_base[s] + t
            toff = 128 * gt
            xw = 2 * PIX if need[s] else PIX      # [ind | lg] or [lg]
            lgo = PIX if need[s] else 0
            rhs_pix = pixrhs[:, s * PIX:(s + 1) * PIX]

            if t == 0:
                pool = pXa if s == 0 else pXb
                Xp[s] = pool.tile([128, 2 * PIX], f32,
                                  tag="Xpa" if s == 0 else "Xpb",
                                  name=f"Xp{s}")[:, 0:xw]
                ipool = pimgA if s == 0 else pimgB
                imgT[s] = ipool.tile([30, PIX], f32,
                                     tag="imgA" if s == 0 else "imgB",
                                     name=f"imgT{s}")

            q = pq.tile([128, PIX], f32, tag="q")
            nc.tensor.matmul(q, pcoef[:, toff:toff + 128], rhs_pix,
                             start=True, stop=True)
            w = wrk.tile([128, PIX], f32, tag="w", name=f"w{s}_{t}")
            nc.scalar.activation(w, q, Act.Relu)
            X = xpool.tile([128, xw], f32r, tag="X", name=f"X{s}_{t}")
            if need[s]:
                nc.vector.tensor_scalar(X[:, 0:PIX], q, 0.0, None, Alu.is_gt)
            nc.scalar.activation(X[:, lgo:lgo + PIX], w, Act.Ln,
                                 bias=1.0, scale=float(-(1.0 - EPS) / R2))
            wbuf[(s, t)] = w
            Xbuf[(s, t)] = X

        def emit_b(s, t):
            nt = ntiles[s]
            gt = tile_base[s] + t
            lgo = PIX if need[s] else 0
            w = wbuf.pop((s, t))
            X = Xbuf.pop((s, t))

            # strict-upper cumsum: exclusive coverage count and exclusive
            # log-transmittance (plus the carry already in the bank).
            # One matmul per 512-wide PSUM bank (a single matmul may not
            # span two banks).
            if need[s]:
                nc.tensor.matmul(Xp[s][:, 0:PIX], triu1[:], X[:, 0:PIX],
                                 start=(t == 0), stop=(t == nt - 1),
                                 skip_group_check=True)
            nc.tensor.matmul(Xp[s][:, lgo:lgo + PIX], triu1[:],
                             X[:, lgo:lgo + PIX],
                             start=(t == 0), stop=(t == nt - 1),
                             skip_group_check=True)
            Tr = wrk.tile([128, PIX], f32, tag="Tr")
            nc.scalar.activation(Tr, Xp[s][:, lgo:lgo + PIX], Act.Exp)
            t1 = wrk.tile([128, PIX], f32r, tag="t1")
            nc.gpsimd.tensor_tensor(t1, w, Tr, Alu.mult)
            if need[s]:
                wT = wrk.tile([128, PIX], f32r, tag="wT")
                nc.vector.scalar_tensor_tensor(wT, Xp[s][:, 0:PIX],
                                               float(KSEL) - 0.5, t1,
                                               Alu.is_lt, Alu.mult)
            else:
                wT = t1
            nc.tensor.matmul(imgT[s][:], feats[:, gt, :],
                             wT[:], start=(t == 0), stop=(t == nt - 1),
                             skip_group_check=True)
            if t < nt - 1:
                # promote the bank to the next tile's carry:
                # += tril0 @ X makes every row hold the full column sum
                # (triu1 + tril0 == ones)
                if need[s]:
                    nc.tensor.matmul(Xp[s][:, 0:PIX], tril0[:], X[:, 0:PIX],
                                     start=False, stop=False,
                                     skip_group_check=True)
                nc.tensor.matmul(Xp[s][:, lgo:lgo + PIX], tril0[:],
                                 X[:, lgo:lgo + PIX],
                                 start=False, stop=False,
                                 skip_group_check=True)
            if t == nt - 1:
                emit_epilogue(s)

        # run phase A a few tiles ahead of phase B so the PE queue stays
        # dense (ramps the p-state) and never stalls on Scalar/DVE
        DELAY = 4
        for i, (s, t) in enumerate(sched):
            emit_a(s, t)
            if i >= DELAY:
                emit_b(*sched[i - DELAY])
        for i in range(max(0, len(sched) - DELAY), len(sched)):
            emit_b(*sched[i])

        nc.sync.dma_start(
            out=d_out[:],
            in_=outsb.rearrange("p s g c -> p (s g c)"))

    # The act-table placement pass picks a table per activation locally,
    # flapping between the ln-only and exp-only sets (one 1.3us load per
    # flap).  ln and exp share the natural_log_exp_and_others set; hide
    # them from every other set during placement so the pass settles on
    # the shared one (walrus still loads the full real table contents).
    orig_get = hw_specs.get_activation_tables

    def _pinned(arch):
        tabs = orig_get(arch)
        out = {}
        for name, funcs in tabs.items():
            if name != "natural_log_exp_and_others":
                funcs = {f for f in funcs
                         if f.name.lower() not in ("ln", "exp")}
            out[name] = funcs
        return out

    # bacc binds get_activation_tables via from-import; patch both
    hw_specs.get_activation_tables = _pinned
    bacc.get_activation_tables = _pinned
    try:
        nc.compile()
    finally:
        hw_specs.get_activation_tables = orig_get
        bacc.get_activation_tables = orig_get
    return nc


def kernel(vertsparam, sh_param, viewdir, cam_R, cam_T, _trace=False):
    from concourse.bass_utils import run_bass_kernel_spmd

    L, need, in_maps, meta = _host_prep(vertsparam, sh_param, viewdir,
                                        cam_R, cam_T)
    key = (L, need)
    if key not in _BUILD_CACHE:
        _BUILD_CACHE[key] = _build(L, need)
    nc = _BUILD_CACHE[key]

    res = run_bass_kernel_spmd(nc, in_maps, core_ids=list(range(NCORES)),
                               trace=_trace)

    image = np.zeros((1, S, S, 3), dtype=np.float32)
    for k in range(NCORES):
        out = res.results[k]["out"].reshape(128, NSLOT, CHROWS, 3)
        for s in range(NSLOT):
            c = meta[k][s]
            for gi in range(CHROWS):
                image[0, CHROWS * c + gi, :, :] = out[:, s, gi, :]
    if _trace:
        kernel._last_exec_time_ns = res.exec_time_ns
        kernel._last_trace = res.instructions_and_trace
    return image


# revision 22
# speedup vs baseline: 1.5995x; 1.2771x over previous
"""Point-cloud rasterization + SH shading kernel for 8 Trainium2 cores.

Algorithm (dense, z-sorted, column-halved jobs):
  - Host: project points (copy) and bin them into 64 jobs = 32 row-chunks
    (4 image rows) x 2 column halves (64 cols); points near the half
    boundary are duplicated into both halves so each pixel's covering set
    stays within one job.  Jobs are z-sorted, ranked by size, and assigned
    to (core, slot) round-robin so all cores compile one program shape.
    The per-pixel SH basis (from normalized viewdir) is precomputed on the
    host, as are rescaled projection/pixel coefficient rows so the device
    computes q = R2 - d2 (the 1/R2 scale folds into activation scales).
  - Device (per core, SPMD): project gathered points (fp32r matmul), then
    per job tile (128 z-sorted points x 256 pixels): q via a K=4 fp32
    matmul, w = relu(q) (Scalar), lg = ln(1-(1-eps)w') (Scalar),
    ind = q>0 (DVE), both exclusive cumulative sums via strict-triangular
    fp32r matmuls into a per-slot PSUM bank whose cross-tile carry is
    accumulated in place with an inclusive-lower-triangular matmul
    (ones = triu1 + tril0).  Then in parallel: m = (cnt<15.5)*w (DVE) and
    Tr = exp(C) (Scalar); wT = m*Tr (GpSimd); composite the 30 channels
    with an fp32r matmul.  Slots are processed as three interleaved
    streams so every engine stays fed.  Slot pairs share a 128-partition
    SH epilogue (two 64-column jobs stacked): DVE 32x32 stream transposes
    to pixel-major, then basis dot products, clip.
  - The count cutoff is exact: for any selected point, all earlier
    covering points are also selected (counts are monotone in z), so lg
    needs no mask and the mask is applied once at the end.
"""

import numpy as np

S = 128
N = 4096
KSEL = 16
RS = 0.03
R2 = RS * RS
F = 2.0
NCORES = 8
CHROWS = 4                   # image rows per chunk
NCHUNK = S // CHROWS         # 32
HALVES = 2
NJOB = NCHUNK * HALVES       # 64
JCOL = S // HALVES           # 64 columns per job
JPIX = CHROWS * JCOL         # 256 pixels per job
NSLOT = NJOB // NCORES       # 8 jobs (slots) per core
PAIRS = ((0, 7), (1, 2), (3, 4), (5, 6))
STREAMS = ((0, 7), (1, 2), (3, 4, 5, 6))
EPS = 1e-6

_C0 = 0.28209479177387814
_C1 = 0.4886025119029199
_C2 = (1.0925484305920792, -1.0925484305920792, 0.31539156525252005,
       -1.0925484305920792, 0.5462742152960396)

_BUILD_CACHE = {}


def _host_prep(vertsparam, sh_param, viewdir, cam_R, cam_T):
    v = np.asarray(vertsparam, dtype=np.float32)
    sh = np.asarray(sh_param, dtype=np.float32)
    vd = np.asarray(viewdir, dtype=np.float32)
    R = np.asarray(cam_R, dtype=np.float32)
    T = np.asarray(cam_T, dtype=np.float32)

    # host-side projection copy, used only for binning / sorting decisions
    cam = (v @ R + T).astype(np.float32)
    z = cam[:, 2]
    with np.errstate(divide="ignore", invalid="ignore"):
        x = (F * cam[:, 0] / z).astype(np.float32)
        y = (F * cam[:, 1] / z).astype(np.float32)

    order = np.argsort(z, kind="stable")
    zs, xs, ys = z[order], x[order], y[order]

    g = (1.0 - (2.0 * np.arange(S) + 1.0) / S).astype(np.float32)

    job_lists = []
    job_maxcov = []
    for c in range(NCHUNK):
        rows = np.arange(CHROWS * c, CHROWS * c + CHROWS)
        pys = -g[rows]
        ysel = (zs > 0) & (ys >= pys.min() - RS) & (ys <= pys.max() + RS)
        for h in range(HALVES):
            cols = np.arange(JCOL * h, JCOL * h + JCOL)
            pxs = g[cols]
            sel = ysel & (xs >= pxs.min() - RS) & (xs <= pxs.max() + RS)
            pts = np.where(sel)[0]
            job_lists.append(order[pts])
            if len(pts):
                px = np.tile(pxs, CHROWS)
                py = np.repeat(-g[rows], JCOL)
                d2 = ((px[:, None] - xs[pts][None, :]) ** 2
                      + (py[:, None] - ys[pts][None, :]) ** 2)
                job_maxcov.append(int((d2 < R2).sum(1).max()))
            else:
                job_maxcov.append(0)
    cnts = np.array([len(l) for l in job_lists])

    rank = np.argsort(-cnts, kind="stable")
    slots = [[int(rank[NCORES * s + k]) for k in range(NCORES)]
             for s in range(NSLOT)]
    L = [max(128, int(np.ceil(max(cnts[j] for j in slots[s]) / 128)) * 128)
         for s in range(NSLOT)]
    P_tot = sum(L)
    need = tuple(bool(max(job_maxcov[j] for j in slots[s]) > KSEL)
                 for s in range(NSLOT))

    # pad vertex that projects far off-screen with z=1 (w == 0 everywhere)
    cam_pad = np.array([1e4, 1e4, 1.0], dtype=np.float64)
    v_pad = np.linalg.solve(R.astype(np.float64).T,
                            (cam_pad - T.astype(np.float64))).astype(np.float32)

    def job_basis(j):
        c, h = divmod(j, HALVES)
        rows = np.arange(CHROWS * c, CHROWS * c + CHROWS)
        cols = np.arange(JCOL * h, JCOL * h + JCOL)
        d = vd[rows][:, cols].transpose(1, 0, 2).astype(np.float64)  # [64,4,3]
        d = d / np.linalg.norm(d, axis=-1, keepdims=True)
        dx, dy, dz = d[..., 0], d[..., 1], d[..., 2]
        B = np.stack([
            -_C1 * dy, _C1 * dz, -_C1 * dx,
            _C2[0] * dx * dy, _C2[1] * dy * dz,
            _C2[2] * (2.0 * dz * dz - dx * dx - dy * dy),
            _C2[3] * dx * dz, _C2[4] * (dx * dx - dy * dy),
        ], axis=-1)  # [64, 4, 8]
        return np.concatenate(
            [B, np.zeros((JCOL, CHROWS, 1))], axis=-1)  # [64, 4, 9]

    in_maps = []
    meta = []  # per core: job id per slot
    for k in range(NCORES):
        verts_g = np.tile(v_pad, (P_tot, 1)).astype(np.float32)
        feats_g = np.zeros((P_tot, 30), dtype=np.float32)
        pixrhs = np.zeros((4, NSLOT * JPIX), dtype=np.float32)
        basis = np.zeros((128, len(PAIRS) * CHROWS * 9), dtype=np.float32)
        bview = basis.reshape(128, len(PAIRS), CHROWS, 9)
        off = 0
        jobs_k = []
        for s in range(NSLOT):
            j = slots[s][k]
            jobs_k.append(j)
            pts = job_lists[j]
            n = len(pts)
            verts_g[off:off + n] = v[pts]
            feats_g[off:off + n] = sh[pts]
            c, h = divmod(j, HALVES)
            rows = np.arange(CHROWS * c, CHROWS * c + CHROWS)
            cols = np.arange(JCOL * h, JCOL * h + JCOL)
            px = np.tile(g[cols], CHROWS)
            py = np.repeat(-g[rows], JCOL)
            pixrhs[0, s * JPIX:(s + 1) * JPIX] = px
            pixrhs[1, s * JPIX:(s + 1) * JPIX] = py
            pixrhs[2, s * JPIX:(s + 1) * JPIX] = -(px * px + py * py)
            pixrhs[3, s * JPIX:(s + 1) * JPIX] = 1.0
            off += L[s]
        for p, (sa, sb) in enumerate(PAIRS):
            bview[0:JCOL, p] = job_basis(slots[sa][k]).astype(np.float32)
            bview[JCOL:128, p] = job_basis(slots[sb][k]).astype(np.float32)
        in_maps.append({
            "vertsT": np.ascontiguousarray(verts_g.T),        # [3, P_tot]
            "feats": np.ascontiguousarray(feats_g),           # [P_tot, 30]
            "pixrhs": np.ascontiguousarray(pixrhs),           # [4, 2048]
            "basis": np.ascontiguousarray(basis),             # [128, 144]
            "camR": np.ascontiguousarray(R),                  # [3, 3]
            "camTc": np.ascontiguousarray(T.reshape(3, 1)),   # [3, 1]
            "triu1": np.triu(np.ones((128, 128), dtype=np.float32), 1),
            "tril0": np.tril(np.ones((128, 128), dtype=np.float32), 0),
        })
        meta.append(jobs_k)
    return tuple(L), need, in_maps, meta


def _build(L, need):
    from contextlib import ExitStack

    import concourse.bacc as bacc
    import concourse.bass as bass
    import concourse.hw_specs as hw_specs
    import concourse.tile as tile
    from concourse import mybir

    f32 = mybir.dt.float32
    f32r = mybir.dt.float32r
    Act = mybir.ActivationFunctionType
    Alu = mybir.AluOpType

    P_tot = sum(L)
    ntiles = [l // 128 for l in L]
    tile_base = np.cumsum([0] + ntiles).tolist()
    NT = sum(ntiles)
    PF = P_tot // 128

    nc = bacc.Bacc(None, target_bir_lowering=False)

    d_vertsT = nc.dram_tensor("vertsT", [3, P_tot], f32r, kind="ExternalInput")
    d_feats = nc.dram_tensor("feats", [P_tot, 30], f32r, kind="ExternalInput")
    d_pixrhs = nc.dram_tensor("pixrhs", [4, NSLOT * JPIX], f32,
                              kind="ExternalInput")
    d_basis = nc.dram_tensor("basis", [128, len(PAIRS) * CHROWS * 9], f32,
                             kind="ExternalInput")
    d_R = nc.dram_tensor("camR", [3, 3], f32r, kind="ExternalInput")
    d_Tc = nc.dram_tensor("camTc", [3, 1], f32, kind="ExternalInput")
    d_triu1 = nc.dram_tensor("triu1", [128, 128], f32r, kind="ExternalInput")
    d_tril0 = nc.dram_tensor("tril0", [128, 128], f32r, kind="ExternalInput")
    d_out = nc.dram_tensor("out", [128, len(PAIRS) * CHROWS * 3], f32,
                           kind="ExternalOutput")

    def bcast_free(ap, count):
        # broadcast a [..., 1]-style AP along a new innermost free dim
        return bass.AP(tensor=ap.tensor, offset=ap.offset,
                       ap=list(ap.ap) + [[0, count]])

    with tile.TileContext(nc) as tc, ExitStack() as ctx:
        consts = ctx.enter_context(tc.tile_pool(name="consts", bufs=1))

        vertsT = consts.tile([3, P_tot], f32r)
        nc.sync.dma_start(out=vertsT, in_=d_vertsT[:])
        R_sb = consts.tile([3, 3], f32r)
        nc.sync.dma_start(out=R_sb, in_=d_R[:])
        T_sb = consts.tile([3, 1], f32)
        nc.sync.dma_start(out=T_sb, in_=d_Tc[:])
        pixrhs = consts.tile([4, NSLOT * JPIX], f32)
        nc.sync.dma_start(out=pixrhs, in_=d_pixrhs[:])
        triu1 = consts.tile([128, 128], f32r)
        nc.sync.dma_start(out=triu1, in_=d_triu1[:])
        tril0 = consts.tile([128, 128], f32r)
        nc.sync.dma_start(out=tril0, in_=d_tril0[:])
        feats = consts.tile([128, NT, 30], f32r)
        nc.sync.dma_start(
            out=feats, in_=d_feats.rearrange("(t p) c -> p t c", p=128))
        basis = consts.tile([128, len(PAIRS), CHROWS, 9], f32)
        nc.sync.dma_start(out=basis, in_=d_basis[:].rearrange(
            "p (q g b) -> p q g b", q=len(PAIRS), g=CHROWS))

        # ---- projection: camT = R^T @ vertsT + T, then pcoef rows ----
        # Compute engines may only address partition offsets {0,32,64,96},
        # so row quantities live in [128, PF] layout; DMA (which has no such
        # restriction) extracts camT rows and assembles pcoef.
        # pcoef rows: {2*x_ndc, 2*y_ndc, 1, R2 - x_ndc^2 - y_ndc^2} so that
        # q = pcoef . {px, py, -(px^2+py^2), 1} = R2 - d2.
        camT = consts.tile([3, P_tot], f32)
        pcoef = consts.tile([4, P_tot], f32)
        xrow = consts.tile([128, PF], f32)
        yrow = consts.tile([128, PF], f32)
        zrow = consts.tile([128, PF], f32)
        rz = consts.tile([128, PF], f32)
        r0 = consts.tile([128, PF], f32)
        r1 = consts.tile([128, PF], f32)
        s0 = consts.tile([128, PF], f32)
        s1 = consts.tile([128, PF], f32)
        onesrow = consts.tile([128, PF], f32)
        nc.vector.memset(onesrow, 1.0)

        with tc.tile_pool(name="pproj", bufs=2, space="PSUM") as pproj:
            nchk = (P_tot + 511) // 512
            for i in range(nchk):
                a, b = 512 * i, min(512 * (i + 1), P_tot)
                pt = pproj.tile([3, 512], f32)
                nc.tensor.matmul(pt[:, :b - a], R_sb[:],
                                 vertsT[:, a:b], start=True, stop=True)
                nc.vector.tensor_scalar(camT[:, a:b], pt[:, :b - a],
                                        T_sb[:], None, Alu.add)

        nc.sync.dma_start(out=xrow, in_=camT[0:1, :])
        nc.sync.dma_start(out=yrow, in_=camT[1:2, :])
        nc.sync.dma_start(out=zrow, in_=camT[2:3, :])
        nc.vector.reciprocal(rz, zrow)
        nc.vector.tensor_scalar(xrow, xrow, float(2.0 * F), None, Alu.mult)
        nc.vector.tensor_scalar(yrow, yrow, float(2.0 * F), None, Alu.mult)
        nc.vector.tensor_mul(r0, xrow, rz)      # 2*x_ndc
        nc.vector.tensor_mul(r1, yrow, rz)      # 2*y_ndc
        nc.vector.tensor_mul(s0, r0, r0)
        nc.vector.tensor_mul(s1, r1, r1)
        nc.vector.tensor_add(s0, s0, s1)        # 4*(xn^2+yn^2)
        nc.vector.tensor_scalar(s0, s0, -0.25, float(R2), Alu.mult, Alu.add)
        nc.sync.dma_start(out=pcoef[0:1, :], in_=r0)
        nc.sync.dma_start(out=pcoef[1:2, :], in_=r1)
        nc.sync.dma_start(out=pcoef[2:3, :], in_=onesrow)
        nc.sync.dma_start(out=pcoef[3:4, :], in_=s0)

        outsb = consts.tile([128, len(PAIRS), CHROWS, 3], f32)

        wrk = ctx.enter_context(tc.tile_pool(name="wrk", bufs=6))
        xpool = ctx.enter_context(tc.tile_pool(name="xpool", bufs=6))
        shp = ctx.enter_context(tc.tile_pool(name="shp", bufs=2))
        sidep = ctx.enter_context(tc.tile_pool(name="sidep", bufs=4))
        tmpp = ctx.enter_context(tc.tile_pool(name="tmpp", bufs=4))
        pq = ctx.enter_context(tc.tile_pool(name="pq", bufs=2, space="PSUM"))
        pX = [ctx.enter_context(tc.tile_pool(name=f"pX{i}", bufs=1,
                                             space="PSUM"))
              for i in range(len(STREAMS))]
        pimg = [ctx.enter_context(tc.tile_pool(name=f"pimg{i}", bufs=1,
                                               space="PSUM"))
                for i in range(len(STREAMS))]
        stream_of = {}
        for i, st in enumerate(STREAMS):
            for s in st:
                stream_of[s] = i
        pair_of = {}
        for p, (sa, sb) in enumerate(PAIRS):
            pair_of[sa] = (p, 0)
            pair_of[sb] = (p, 1)

        Xp = {}
        imgT = {}
        wbuf = {}
        Xbuf = {}
        imgTs_of = {}

        def emit_pair_sh(p):
            sa, sb = PAIRS[p]
            img30 = shp.tile([128, CHROWS, 32], f32, tag="img30")
            for side, s in ((0, sa), (1, sb)):
                v32 = imgTs_of.pop(s)[:, :].rearrange(
                    "p (g j c) -> p g j c", g=CHROWS, j=2)
                for jb in range(2):
                    o = 64 * side + 32 * jb
                    nc.vector.transpose(img30[o:o + 32], v32[:, :, jb, :])
            sh30 = img30[:, :, 0:30].rearrange("p g (b c) -> p g b c", b=10)
            acc = shp.tile([128, CHROWS, 3], f32, tag="acc")
            nc.vector.scalar_tensor_tensor(acc, sh30[:, :, 1, :], float(_C0),
                                           sh30[:, :, 0, :], Alu.mult, Alu.add)
            for b in range(8):
                tmp = tmpp.tile([128, CHROWS, 3], f32, tag=f"tmp{b}")
                nc.vector.tensor_tensor(tmp, sh30[:, :, b + 2, :],
                                        bcast_free(basis[:, p, :, b], 3),
                                        Alu.mult)
                nc.vector.tensor_add(acc, acc, tmp)
            nc.vector.tensor_scalar(outsb[:, p], acc, 0.0, 1.0,
                                    Alu.max, Alu.min)

        def emit_a(s, t):
            gt = tile_base[s] + t
            toff = 128 * gt
            xw = 2 * JPIX if need[s] else JPIX     # [cnt | lg] or [lg]
            rhs_pix = pixrhs[:, s * JPIX:(s + 1) * JPIX]

            if t == 0:
                i = stream_of[s]
                Xp[s] = pX[i].tile([128, 2 * JPIX], f32, tag=f"Xp{i}",
                                   name=f"Xp{s}")[:, 0:xw]
                imgT[s] = pimg[i].tile([30, JPIX], f32, tag=f"img{i}",
                                       name=f"imgT{s}")

            q = pq.tile([128, JPIX], f32, tag="q")
            nc.tensor.matmul(q, pcoef[:, toff:toff + 128], rhs_pix,
                             start=True, stop=True)
            w = wrk.tile([128, JPIX], f32, tag="w", name=f"w{s}_{t}")
            nc.scalar.activation(w, q, Act.Relu)
            X = xpool.tile([128, xw], f32r, tag="X", name=f"X{s}_{t}")
            if need[s]:
                nc.vector.tensor_scalar(X[:, 0:JPIX], q, 0.0, None, Alu.is_gt)
            lgo = JPIX if need[s] else 0
            nc.scalar.activation(X[:, lgo:lgo + JPIX], w, Act.Ln,
                                 bias=1.0, scale=float(-(1.0 - EPS) / R2))
            wbuf[(s, t)] = w
            Xbuf[(s, t)] = X

        def emit_b(s, t):
            nt = ntiles[s]
            gt = tile_base[s] + t
            lgo = JPIX if need[s] else 0
            w = wbuf.pop((s, t))
            X = Xbuf.pop((s, t))

            # strict-upper cumsum: exclusive coverage count and exclusive
            # log-transmittance (plus the carry already in the bank)
            if need[s]:
                nc.tensor.matmul(Xp[s][:, 0:JPIX], triu1[:], X[:, 0:JPIX],
                                 start=(t == 0), stop=(t == nt - 1),
                                 skip_group_check=True)
            nc.tensor.matmul(Xp[s][:, lgo:lgo + JPIX], triu1[:],
                             X[:, lgo:lgo + JPIX],
                             start=(t == 0), stop=(t == nt - 1),
                             skip_group_check=True)
            Tr = wrk.tile([128, JPIX], f32, tag="Tr")
            nc.scalar.activation(Tr, Xp[s][:, lgo:lgo + JPIX], Act.Exp)
            if need[s]:
                m = wrk.tile([128, JPIX], f32, tag="m")
                nc.vector.scalar_tensor_tensor(m, Xp[s][:, 0:JPIX],
                                               float(KSEL) - 0.5, w,
                                               Alu.is_lt, Alu.mult)
            else:
                m = w
            wT = wrk.tile([128, JPIX], f32r, tag="wT")
            nc.gpsimd.tensor_tensor(wT, m, Tr, Alu.mult)
            nc.tensor.matmul(imgT[s][:], feats[:, gt, :],
                             wT[:], start=(t == 0), stop=(t == nt - 1),
                             skip_group_check=True)
            if t < nt - 1:
                # promote the bank to the next tile's carry:
                # += tril0 @ X makes every row hold the full column sum
                # (triu1 + tril0 == ones)
                if need[s]:
                    nc.tensor.matmul(Xp[s][:, 0:JPIX], tril0[:], X[:, 0:JPIX],
                                     start=False, stop=False,
                                     skip_group_check=True)
                nc.tensor.matmul(Xp[s][:, lgo:lgo + JPIX], tril0[:],
                                 X[:, lgo:lgo + JPIX],
                                 start=False, stop=False,
                                 skip_group_check=True)
            if t == nt - 1:
                p, side = pair_of[s]
                imgTs = sidep.tile([32, JPIX], f32, tag=f"imgTs{side}",
                                   name=f"imgTs{s}")
                # w carries a factor R2 (q = R2 - d2); undo it here
                nc.scalar.activation(imgTs[0:30, :], imgT[s][:], Act.Copy,
                                     scale=float(1.0 / R2))
                imgTs_of[s] = imgTs
                other = PAIRS[p][1 - side]
                if other in imgTs_of:
                    emit_pair_sh(p)

        # merge the three streams round-robin at tile granularity; phase A
        # runs a few tiles ahead of phase B so the PE queue stays dense
        lists = [[(s, t) for s in st for t in range(ntiles[s])]
                 for st in STREAMS]
        sched = []
        idx = [0] * len(lists)
        while any(idx[i] < len(lists[i]) for i in range(len(lists))):
            for i in range(len(lists)):
                if idx[i] < len(lists[i]):
                    sched.append(lists[i][idx[i]])
                    idx[i] += 1
        DELAY = 4
        for i, (s, t) in enumerate(sched):
            emit_a(s, t)
            if i >= DELAY:
                emit_b(*sched[i - DELAY])
        for i in range(max(0, len(sched) - DELAY), len(sched)):
            emit_b(*sched[i])

        nc.sync.dma_start(
            out=d_out[:],
            in_=outsb.rearrange("p q g c -> p (q g c)"))

    # The act-table placement pass picks a table per activation locally,
    # flapping between the ln-only and exp-only sets (one 1.3us load per
    # flap).  ln and exp share the natural_log_exp_and_others set; hide
    # them from every other set during placement so the pass settles on
    # the shared one (walrus still loads the full real table contents).
    orig_get = hw_specs.get_activation_tables

    def _pinned(arch):
        tabs = orig_get(arch)
        out = {}
        for name, funcs in tabs.items():
            if name != "natural_log_exp_and_others":
                funcs = {f for f in funcs
                         if f.name.lower() not in ("ln", "exp")}
            out[name] = funcs
        return out

    # bacc binds get_activation_tables via from-import; patch both
    hw_specs.get_activation_tables = _pinned
    bacc.get_activation_tables = _pinned
    try:
        nc.compile()
    finally:
        hw_specs.get_activation_tables = orig_get
        bacc.get_activation_tables = orig_get
    return nc


def kernel(vertsparam, sh_param, viewdir, cam_R, cam_T, _trace=False):
    from concourse.bass_utils import run_bass_kernel_spmd

    L, need, in_maps, meta = _host_prep(vertsparam, sh_param, viewdir,
                                        cam_R, cam_T)
    key = (L, need)
    if key not in _BUILD_CACHE:
        _BUILD_CACHE[key] = _build(L, need)
    nc = _BUILD_CACHE[key]

    res = run_bass_kernel_spmd(nc, in_maps, core_ids=list(range(NCORES)),
                               trace=_trace)

    image = np.zeros((1, S, S, 3), dtype=np.float32)
    for k in range(NCORES):
        out = res.results[k]["out"].reshape(128, len(PAIRS), CHROWS, 3)
        for p, (sa, sb) in enumerate(PAIRS):
            for side, s in ((0, sa), (1, sb)):
                j = meta[k][s]
                c, h = divmod(j, HALVES)
                for gi in range(CHROWS):
                    image[0, CHROWS * c + gi, JCOL * h:JCOL * (h + 1), :] = \
                        out[64 * side:64 * side + JCOL, p, gi, :]
    if _trace:
        kernel._last_exec_time_ns = res.exec_time_ns
        kernel._last_trace = res.instructions_and_trace
    return image
